# revision 14
# baseline (speedup 1.0000x reference)
"""MLA prefill kernel for Trainium2, batch x head-group parallel on 8 cores.

Sharding: 8 cores = 4 batches x 2 head-groups. Each core runs ONE batch
(2048 rows) with 8 of the 16 heads, so the replicated wkv_a projection is
computed 2x (not 8x as in a pure head split) -- the single biggest PE-cycle
saving vs the head-only sharding. Each core emits a partial output
projection [D, S] (transposed, fp16); host sums head-group pairs.

Per-core fully fused, software-pipelined schedule (row-block 512 =
attention q-block):
- fp16 datapath (better mantissa than bf16, enables DVE 2x fast modes),
  fp32 PSUM accumulation, fp16 softmax accumulators.
- The attention inner loop is ACT-bound (exp); the next block's kv/k_nope/v
  projection chains are interleaved between attention k-tile groups so the
  tensor engine fills those gaps. q-projection chains run at block
  boundaries, giving ACT a window to drain its exp backlog.
- wq (6MB) and wo (4MB) don't fit in SBUF next to per-batch K/V for 8
  heads; both stream through small chunk tiles in consumption order.
- Attention runs two 4-head passes per q-block (PSUM: 2 proj + 2 score +
  4 PV-accumulate banks = 8). Rope K=64 score matmuls are issued as
  back-to-back partition-half pairs -> concurrent PE row-groups.
- Causal masking: one [128,512] (c>=p) tile serves every diagonal
  sub-tile at any offset; dead columns are sliced out of exp/PV/rowsum
  (single per-head fp16 accumulator, no memsets needed).
- rsqrt = exp(-0.5*ln(y)) keeps one ACT table set.

Contract: kernel(**inputs) takes FULL unsharded inputs, returns FULL
[B,S,D] float32.
"""
import sys
sys.path.insert(0, '/opt/trn_rl_repo')
import numpy as np
from collections import deque

B, S, D = 4, 2048, 2048
H = 16
NOPE, ROPE, VD = 128, 64, 128
QK = NOPE + ROPE
KVR = 512
THETA = 10000.0
EPS = 1e-6
SCALE = QK ** -0.5
NCORES = 8
HPC = 8                    # heads per core
RB = 512                   # row block == attention q block
NRB = S // RB              # 4 blocks per core (one batch)
NQC = 12                   # wq out chunks: 8 nope + 4 pe-pair
NWQ = NRB * NQC            # total wq chunk loads

_cache = {}


def _build_nc():
    import concourse.bass as bass  # noqa: F401
    import concourse.mybir as mybir
    import concourse.tile as tile
    from concourse import bacc
    from contextlib import ExitStack

    F32 = mybir.dt.float32
    F32R = mybir.dt.float32r
    F16 = mybir.dt.float16
    EXP = mybir.ActivationFunctionType.Exp
    LN = mybir.ActivationFunctionType.Ln
    MULT = mybir.AluOpType.mult
    ADD = mybir.AluOpType.add

    nc = bacc.Bacc("TRN2", target_bir_lowering=False, debug=False)

    xT = nc.dram_tensor("xT", [D, S], F16, kind="ExternalInput")
    wqT = nc.dram_tensor("wqT", [128, NQC, 16, 128], F16, kind="ExternalInput")
    wkvT = nc.dram_tensor("wkvT", [D, KVR + 2 * ROPE], F16, kind="ExternalInput")
    wbkT = nc.dram_tensor("wbkT", [KVR, HPC * NOPE], F16, kind="ExternalInput")
    wbvT = nc.dram_tensor("wbvT", [KVR, HPC * VD], F16, kind="ExternalInput")
    woT = nc.dram_tensor("woT", [128, 16, HPC, 128], F16, kind="ExternalInput")
    ropeC = nc.dram_tensor("ropeC", [128, S], F16, kind="ExternalInput")
    ropeS = nc.dram_tensor("ropeS", [128, S], F16, kind="ExternalInput")
    perm = nc.dram_tensor("perm", [128, 128], F16, kind="ExternalInput")
    ones16 = nc.dram_tensor("ones16", [128, 128], F16, kind="ExternalInput")
    onesw = nc.dram_tensor("onesw", [128, 128], F32R, kind="ExternalInput")
    mask0 = nc.dram_tensor("mask0", [128, RB], F16, kind="ExternalInput")
    pout = nc.dram_tensor("pout", [D, S], F16, kind="ExternalOutput")
    pout_r = pout.ap().rearrange("(dg ci) s -> ci dg s", ci=128).rearrange(
        "ci (dq df) s -> ci dq df s", df=2)

    xT_r = xT.ap().rearrange("(co ci) r -> ci co r", ci=128)       # [128,16,S]
    wkvT_r = wkvT.ap().rearrange("(co ci) f -> ci co f", ci=128)   # [128,16,640]
    wbkT_r = wbkT.ap().rearrange("(co ci) f -> ci co f", ci=128)   # [128,4,1024]
    wbvT_r = wbvT.ap().rearrange("(co ci) f -> ci co f", ci=128)

    with tile.TileContext(nc) as tc:
        with ExitStack() as stk:
            gpool = stk.enter_context(tc.tile_pool(name="gconst", bufs=1))
            wp = stk.enter_context(tc.tile_pool(name="pw", bufs=1))
            wqp = stk.enter_context(tc.tile_pool(name="pwq", bufs=2))
            wop = stk.enter_context(tc.tile_pool(name="pwo", bufs=3))
            xp = stk.enter_context(tc.tile_pool(name="px", bufs=1))
            qp_ = stk.enter_context(tc.tile_pool(name="pq", bufs=2))
            sp = stk.enter_context(tc.tile_pool(name="p1sb", bufs=2))
            ep3 = stk.enter_context(tc.tile_pool(name="p3ex", bufs=3))
            sp3 = stk.enter_context(tc.tile_pool(name="p3sb", bufs=1))
            op3 = stk.enter_context(tc.tile_pool(name="p3o", bufs=1))
            sp4 = stk.enter_context(tc.tile_pool(name="p4sb", bufs=2))
            psA = stk.enter_context(tc.tile_pool(name="psproj", bufs=2, space="PSUM"))
            pss = stk.enter_context(tc.tile_pool(name="pssc", bufs=2, space="PSUM"))
            pso = stk.enter_context(tc.tile_pool(name="psacc", bufs=4, space="PSUM"))

            # ---- constants / persistent tiles ----
            perm_sb = gpool.tile([128, 128], F16)
            ones_sb = gpool.tile([128, 128], F16)
            onesr_sb = gpool.tile([128, 128], F32R)
            eps_sb = gpool.tile([128, 1], F32)
            mask_sb = gpool.tile([128, RB], F16)
            kn3 = gpool.tile([128, HPC, S], F16)        # [nope, h, kpos]
            v3 = gpool.tile([128, 16, HPC, VD], F16)    # [kpos128, ktile, h, vd]
            kp3 = gpool.tile([128, S], F16)             # k_pe dup'd halves

            wkv_sb = wp.tile([128, 16, 640], F16)
            wbk_sb = wp.tile([128, 4, HPC * NOPE], F16)
            wbv_sb = wp.tile([128, 4, HPC * VD], F16)
            ropeC_sb = wp.tile([128, S], F16)
            ropeS_sb = wp.tile([128, S], F16)

            # x row-block tile: single-buffered; next block's DMA is issued
            # after the q chains (this block's last readers) in program order.
            xt = xp.tile([128, 16, RB], F16, tag="xt", name="xt0")
            nc.sync.dma_start(xt, xT_r[:, :, 0:RB])

            # wq chunk stream: chunk g = rb*12 + j, double-buffered
            wq_pend = deque()

            def queue_wq(g):
                t = wqp.tile([128, 16, 128], F16, tag="wqc", name=f"wqc{g}")
                j = g % NQC
                nc.sync.dma_start(t, wqT.ap()[:, j])
                wq_pend.append(t)

            queue_wq(0)
            queue_wq(1)

            nc.vector.memset(eps_sb, EPS)
            nc.scalar.dma_start(perm_sb, perm.ap())
            nc.scalar.dma_start(ones_sb, ones16.ap())
            nc.scalar.dma_start(onesr_sb, onesw.ap())
            nc.scalar.dma_start(mask_sb, mask0.ap())
            nc.scalar.dma_start(ropeC_sb, ropeC.ap())
            nc.scalar.dma_start(ropeS_sb, ropeS.ap())

            def bulk_preamble():
                nc.scalar.dma_start(wkv_sb, wkvT_r)
                nc.scalar.dma_start(wbk_sb, wbkT_r)
                nc.scalar.dma_start(wbv_sb, wbvT_r)

            # wo chunk stream: chunk g = qb*16 + dt, 4 bufs
            wo_pend = deque()

            def queue_wo(g):
                t = wop.tile([128, HPC, 128], F16, tag="woc", name=f"woc{g}")
                dt_ = g % 16
                nc.sync.dma_start(t, woT.ap()[:, dt_])
                wo_pend.append(t)

            for g in range(3):
                queue_wo(g)

            def q_chains(rb, qn, qp3):
                """q projection for row-block rb (last readers of xt[rb])."""
                sl = rb * RB
                pend_pe = []
                for j in range(NQC):
                    g = rb * NQC + j
                    wqc = wq_pend.popleft()
                    ps_q = psA.tile([128, RB], F32, tag="proj", name="ps_q")
                    for dc in range(16):
                        nc.tensor.matmul(ps_q, wqc[:, dc, :], xt[:, dc, :],
                                         start=(dc == 0), stop=(dc == 15))
                    if g + 2 < NWQ:
                        queue_wq(g + 2)
                    if pend_pe:
                        pend_pe.pop(0)()
                    if j < 8:
                        if j % 2 == 0:
                            nc.scalar.copy(qn[:, j, :], ps_q)
                        else:
                            nc.vector.tensor_copy(qn[:, j, :], ps_q)
                    else:
                        jj = j - 8
                        qpe_sb = sp.tile([128, RB], F16, tag="pe", name="qpe_sb")
                        nc.scalar.copy(qpe_sb, ps_q)

                        def rope_q(jj=jj, qpe_sb=qpe_sb):
                            ps_qs = pss.tile([128, RB], F32, tag="s", name="ps_qs")
                            nc.tensor.matmul(ps_qs, perm_sb, qpe_sb,
                                             start=True, stop=True)
                            t1 = sp.tile([128, RB], F16, tag="ropt1", bufs=1,
                                         name="t1")
                            nc.vector.tensor_tensor(t1, qpe_sb,
                                                    ropeC_sb[:, sl:sl + RB], MULT)
                            nc.vector.tensor_tensor(qp3[:, jj, :], ps_qs,
                                                    ropeS_sb[:, sl:sl + RB], MULT)
                            nc.vector.tensor_tensor(qp3[:, jj, :], qp3[:, jj, :],
                                                    t1, ADD)
                        pend_pe.append(rope_q)
                for f in pend_pe:
                    f()

            def kv_thunks(rb):
                """kv latent + k_pe + rmsnorm + k_nope + v chains for block rb,
                as a list of closures to interleave into attention."""
                sl = rb * RB
                st = {}
                thunks = []

                def kv_chain(ft):
                    def f():
                        ps_kv = psA.tile([128, RB], F32, tag="proj", name="ps_kv")
                        for dc in range(16):
                            nc.tensor.matmul(ps_kv,
                                             wkv_sb[:, dc, ft * 128:(ft + 1) * 128],
                                             xt[:, dc, :],
                                             start=(dc == 0), stop=(dc == 15))
                        ku = sp.tile([128, RB], F16, tag=f"kvu{ft}", bufs=1,
                                     name=f"ku{ft}")
                        nc.vector.tensor_copy(ku, ps_kv)
                        st.setdefault('kvu', [None] * 4)[ft] = ku
                        if ft == 0:
                            ssacc = sp.tile([128, RB], F32R, tag="ssacc", bufs=1,
                                            name="ssacc")
                            st['ssacc'] = ssacc
                            nc.vector.tensor_tensor(ssacc, ps_kv, ku, MULT)
                        else:
                            sq = sp.tile([128, RB], F16, tag="sq", bufs=1, name="sq")
                            nc.vector.tensor_tensor(sq, ps_kv, ku, MULT)
                            nc.vector.tensor_tensor(st['ssacc'],
                                                    st['ssacc'].bitcast(F32), sq, ADD)
                    return f

                def kpe_chain():
                    ps_kp = psA.tile([128, RB], F32, tag="proj", name="ps_kp")
                    for dc in range(16):
                        nc.tensor.matmul(ps_kp, wkv_sb[:, dc, 512:640],
                                         xt[:, dc, :],
                                         start=(dc == 0), stop=(dc == 15))
                    kpe_sb = sp.tile([128, RB], F16, tag="kpe", name="kpe_sb")
                    nc.scalar.copy(kpe_sb, ps_kp)
                    ps_kps = pss.tile([128, RB], F32, tag="s", name="ps_kps")
                    nc.tensor.matmul(ps_kps, perm_sb, kpe_sb, start=True, stop=True)
                    k1 = sp.tile([128, RB], F16, tag="kropt1", bufs=1, name="k1")
                    nc.vector.tensor_tensor(k1, kpe_sb, ropeC_sb[:, sl:sl + RB], MULT)
                    nc.vector.tensor_tensor(kp3[:, sl:sl + RB], ps_kps,
                                            ropeS_sb[:, sl:sl + RB], MULT)
                    nc.vector.tensor_tensor(kp3[:, sl:sl + RB],
                                            kp3[:, sl:sl + RB], k1, ADD)

                def lam_chain():
                    ps_ms = pss.tile([128, RB], F32, tag="s", name="ps_ms")
                    nc.tensor.matmul(ps_ms, onesr_sb, st['ssacc'], start=True,
                                     stop=True)
                    lam = sp.tile([128, RB], F32, tag="lam", bufs=1, name="lam")
                    nc.scalar.activation(lam, ps_ms, LN, scale=1.0 / KVR, bias=eps_sb)
                    lam2 = sp.tile([128, RB], F16, tag="lam2", bufs=1, name="lam2")
                    nc.scalar.activation(lam2, lam, EXP, scale=-0.5)
                    kvn = []
                    for ft in range(4):
                        kn = sp.tile([128, RB], F16, tag=f"kvn{ft}", bufs=1,
                                     name=f"kvn{ft}")
                        nc.vector.tensor_tensor(kn, st['kvu'][ft], lam2, MULT)
                        kvn.append(kn)
                    st['kvn'] = kvn

                def kn_chain(h):
                    def f():
                        kvn = st['kvn']
                        ps_k = psA.tile([128, RB], F32, tag="proj", name="ps_k")
                        for kc in range(4):
                            nc.tensor.matmul(ps_k,
                                             wbk_sb[:, kc, h * 128:(h + 1) * 128],
                                             kvn[kc], start=(kc == 0), stop=(kc == 3))
                        if h % 2 == 0:
                            nc.vector.tensor_copy(kn3[:, h, sl:sl + RB], ps_k)
                        else:
                            nc.scalar.copy(kn3[:, h, sl:sl + RB], ps_k)
                    return f

                def v_chain(half, rt):
                    def f():
                        kvn = st['kvn']
                        ps_v = psA.tile([128, RB], F32, tag="proj", name="ps_v")
                        for kc in range(4):
                            nc.tensor.matmul(
                                ps_v, kvn[kc][:, rt * 128:(rt + 1) * 128],
                                wbv_sb[:, kc, half * 512:(half + 1) * 512],
                                start=(kc == 0), stop=(kc == 3))
                        ro = rb * 4 + rt
                        if (half + rt) % 2 == 0:
                            nc.vector.tensor_copy(
                                v3[:, ro, half * 4:(half + 1) * 4, :], ps_v)
                        else:
                            nc.scalar.copy(
                                v3[:, ro, half * 4:(half + 1) * 4, :], ps_v)
                    return f

                for ft in range(4):
                    thunks.append(kv_chain(ft))
                thunks.append(lam_chain)
                thunks.append(kpe_chain)
                for h in range(HPC):
                    thunks.append(kn_chain(h))
                for half in range(2):
                    for rt in range(4):
                        thunks.append(v_chain(half, rt))
                return thunks

            def p3_thunks(qb, qn, qp3):
                """Attention thunks (per k-tile group) + output projection."""
                nkt = 4 * qb + 4
                o2 = op3.tile([128, HPC, RB], F16, tag="o2", name="o2")
                thunks = []
                st = {}

                def kt_group(hp, kt):
                    def f():
                        hs = hp * 4
                        if kt == 0:
                            st[hp] = {
                                'ps_o': [pso.tile([128, RB], F32, tag="o",
                                                  name=f"ps_o{i}") for i in range(4)],
                                'acc': [sp3.tile([128, RB], F16, tag=f"acc{i}",
                                                 bufs=1, name=f"acc{i}")
                                        for i in range(4)],
                            }
                        ps_o = st[hp]['ps_o']
                        acc = st[hp]['acc']
                        m = kt - 4 * qb
                        lo = max(0, m) * 128
                        ps_s = [pss.tile([128, RB], F32, tag="s", name=f"ps_s{i}")
                                for i in range(4)]
                        for i in range(4):
                            h = hs + i
                            nc.tensor.matmul(ps_s[i][:, lo:],
                                             kn3[:, h, kt * 128:(kt + 1) * 128],
                                             qn[:, h, lo:],
                                             start=True, stop=False)
                        for i in range(4):
                            h = hs + i
                            hb = (h % 2) * 64
                            nc.tensor.matmul(ps_s[i][:, lo:],
                                             kp3[hb:hb + 64, kt * 128:(kt + 1) * 128],
                                             qp3[hb:hb + 64, h // 2, lo:],
                                             start=False, stop=True)
                        for i in range(4):
                            h = hs + i
                            ex = ep3.tile([128, RB], F16, tag="ex", name="ex")
                            nc.scalar.activation(ex[:, lo:], ps_s[i][:, lo:], EXP)
                            if m >= 0:
                                nc.vector.tensor_tensor(
                                    ex[:, lo:], ex[:, lo:],
                                    mask_sb[:, :RB - lo], MULT)
                            nc.tensor.matmul(ps_o[i][:, lo:], v3[:, kt, h, :],
                                             ex[:, lo:],
                                             start=(kt == 0), stop=(kt == nkt - 1))
                            if kt == 0:
                                nc.vector.tensor_copy(acc[i], ex)
                            else:
                                nc.vector.tensor_tensor(acc[i][:, lo:],
                                                        acc[i][:, lo:],
                                                        ex[:, lo:], ADD)
                    return f

                def rowsum(hp):
                    def f():
                        hs = hp * 4
                        ps_o = st[hp]['ps_o']
                        acc = st[hp]['acc']
                        for i in range(4):
                            ps_r = pss.tile([128, RB], F32, tag="s", name="ps_r")
                            nc.tensor.matmul(ps_r, ones_sb, acc[i], start=True,
                                             stop=True)
                            rec = sp3.tile([128, RB], F16, tag="rec", bufs=2,
                                           name="rec")
                            with nc.allow_low_precision(reason="fp16 softmax recip"):
                                nc.vector.reciprocal(rec, ps_r)
                            nc.vector.tensor_tensor(o2[:, hs + i, :], ps_o[i],
                                                    rec, MULT)
                    return f

                def oproj(dt_):
                    def f():
                        g = qb * 16 + dt_
                        woc = wo_pend.popleft()
                        ps_p = psA.tile([128, RB], F32, tag="proj", name="ps_p")
                        for hc in range(HPC):
                            nc.tensor.matmul(ps_p, woc[:, hc, :], o2[:, hc, :],
                                             start=(hc == 0), stop=(hc == HPC - 1))
                        if g + 3 < 64:
                            queue_wo(g + 3)
                        if dt_ % 2 == 0:
                            st['po'] = sp4.tile([128, 2, RB], F16, tag="po",
                                                bufs=1, name="po")
                        po = st['po']
                        if dt_ % 2 == 0:
                            nc.scalar.copy(po[:, 0, :], ps_p)
                        else:
                            nc.vector.tensor_copy(po[:, 1, :], ps_p)
                        if dt_ % 2 == 1:
                            nc.sync.dma_start(
                                pout_r[:, dt_ // 2, :,
                                       qb * RB:(qb + 1) * RB], po)
                    return f

                for hp in range(2):
                    for kt in range(nkt):
                        thunks.append(kt_group(hp, kt))
                    thunks.append(rowsum(hp))
                for dt_ in range(16):
                    thunks.append(oproj(dt_))
                return thunks

            def interleave(a, b):
                """Emit thunks of a (attention) and b (next-block projections)
                spread evenly; a's order drives correctness, b fills PE."""
                na, nb = len(a), len(b)
                j = 0
                for i, fa in enumerate(a):
                    fa()
                    want = (i + 1) * nb // na
                    while j < want:
                        b[j]()
                        j += 1
                while j < nb:
                    b[j]()
                    j += 1

            # ---- main schedule ----
            pend_kv = kv_thunks(0)
            for rb in range(NRB):
                qn = qp_.tile([128, HPC, RB], F16, tag="qn", name="qn")
                qp3 = qp_.tile([128, 4, RB], F16, tag="qp", name="qp")
                q_chains(rb, qn, qp3)
                if rb == 0:
                    bulk_preamble()
                    for f in pend_kv:
                        f()
                    pend_kv = []
                if rb + 1 < NRB:
                    nc.sync.dma_start(xt,
                                      xT_r[:, :, (rb + 1) * RB:(rb + 2) * RB])
                nxt = kv_thunks(rb + 1) if rb + 1 < NRB else []
                interleave(p3_thunks(rb, qn, qp3), nxt)

    nc.compile()
    return nc


def _prep_inputs(x, wq, wkv_a, kv_norm_w, wkv_b, wo, freqs_cos, freqs_sin):
    f16 = np.float16
    x = np.asarray(x, np.float32)
    wq = np.asarray(wq, np.float32)
    wkv_a = np.asarray(wkv_a, np.float32)
    kv_norm_w = np.asarray(kv_norm_w, np.float32)
    wkv_b = np.asarray(wkv_b, np.float32)
    wo = np.asarray(wo, np.float32)
    cos = np.asarray(freqs_cos, np.float32)   # [S, 32]
    sin = np.asarray(freqs_sin, np.float32)

    C64 = np.repeat(cos.T, 2, axis=0)         # [64, S]
    S64 = np.repeat(sin.T, 2, axis=0).copy()
    S64[0::2] *= -1.0                         # even rows: -sin; odd: +sin
    ropeC = np.ascontiguousarray(np.vstack([C64, C64])).astype(f16)   # [128,S]
    ropeS = np.ascontiguousarray(np.vstack([S64, S64])).astype(f16)

    perm = np.zeros((128, 128), np.float32)
    idx = np.arange(128)
    perm[idx ^ 1, idx] = 1.0                  # out[m] = in[m^1]
    ones = np.ones((128, 128), np.float32)

    # single causal mask tile: mask0[p, c] = 1.0 iff c >= p
    mask0 = (np.arange(RB)[None, :] >= np.arange(128)[:, None]).astype(f16)

    wq_h = wq.reshape(H, QK, D)
    wb_h = (wkv_b * kv_norm_w[None, :]).reshape(H, NOPE + VD, KVR)
    wkv_prep = np.concatenate([wkv_a, wkv_a[KVR:]], axis=0)  # [640, D] dup'd pe

    xT_b = [np.ascontiguousarray(x[b].T).astype(f16) for b in range(B)]
    wkvT = np.ascontiguousarray(wkv_prep.T).astype(f16)

    in_maps = []
    for c in range(NCORES):
        b, hg = c // 2, c % 2
        heads = list(range(hg * HPC, (hg + 1) * HPC))
        # q chunks: 8 per-head nope blocks, then 4 pe pair blocks
        qrows = [wq_h[h, :NOPE] for h in heads]
        for j in range(4):
            qrows.append(wq_h[heads[2 * j], NOPE:])
            qrows.append(wq_h[heads[2 * j + 1], NOPE:])
        wq_prep = np.concatenate(qrows, axis=0) * SCALE          # [1536, D]
        wbk = np.concatenate([wb_h[h, :NOPE] for h in heads], axis=0)   # [1024,512]
        wbv = np.concatenate([wb_h[h, NOPE:] for h in heads], axis=0)
        wo_c = np.concatenate([wo[:, h * VD:(h + 1) * VD] for h in heads],
                              axis=1)                            # [D, 1024]
        in_maps.append({
            "xT": xT_b[b],
            "wqT": np.ascontiguousarray(
                wq_prep.reshape(NQC, 128, 16, 128).transpose(3, 0, 2, 1)
            ).astype(f16),
            "wkvT": wkvT,
            "wbkT": np.ascontiguousarray(wbk.T).astype(f16),
            "wbvT": np.ascontiguousarray(wbv.T).astype(f16),
            "woT": np.ascontiguousarray(
                wo_c.reshape(16, 128, HPC, 128).transpose(3, 0, 2, 1)
            ).astype(f16),
            "ropeC": ropeC,
            "ropeS": ropeS,
            "perm": perm.astype(f16),
            "ones16": ones.astype(f16),
            "onesw": ones.astype(np.float32),
            "mask0": mask0,
        })
    return in_maps


def _get_nc():
    if "nc" not in _cache:
        _cache["nc"] = _build_nc()
    return _cache["nc"]


def kernel(**inputs):
    from concourse.bass_utils import run_bass_kernel_spmd
    nc = _get_nc()
    in_maps = _prep_inputs(**inputs)
    res = run_bass_kernel_spmd(nc, in_maps, core_ids=list(range(NCORES)))
    out = np.empty((B, S, D), np.float32)
    for b in range(B):
        acc = res.results[2 * b]["pout"].astype(np.float32)
        acc += res.results[2 * b + 1]["pout"].astype(np.float32)
        out[b] = acc.T
    return out


# revision 18
# speedup vs baseline: 1.5987x; 1.5987x over previous
"""MLA prefill kernel for Trainium2, batch x head-group parallel on 8 cores.

Sharding: 8 cores = 4 batches x 2 head-groups. Each core runs ONE batch
(2048 rows) with 8 of the 16 heads, so the replicated wkv_a projection is
computed 2x (not 8x as in a pure head split) -- the single biggest PE-cycle
saving vs the head-only sharding. Each core emits a partial output
projection [D, S] (transposed, fp16); host sums head-group pairs.

Per-core fully fused, software-pipelined schedule (row-block 512 =
attention q-block):
- fp16 datapath (better mantissa than bf16, enables DVE 2x fast modes),
  fp32 PSUM accumulation, fp16 softmax accumulators.
- The attention inner loop is ACT-bound (exp); the next block's kv/k_nope/v
  projection chains are interleaved between attention k-tile groups so the
  tensor engine fills those gaps. q-projection chains run at block
  boundaries, giving ACT a window to drain its exp backlog.
- wq (6MB) and wo (4MB) don't fit in SBUF next to per-batch K/V for 8
  heads; both stream through small chunk tiles in consumption order.
- Attention runs two 4-head passes per q-block (PSUM: 2 proj + 2 score +
  4 PV-accumulate banks = 8). Rope K=64 score matmuls are issued as
  back-to-back partition-half pairs -> concurrent PE row-groups.
- Causal masking: one [128,512] (c>=p) tile serves every diagonal
  sub-tile at any offset; dead columns are sliced out of exp/PV/rowsum
  (single per-head fp16 accumulator, no memsets needed).
- rsqrt = exp(-0.5*ln(y)) keeps one ACT table set.

Contract: kernel(**inputs) takes FULL unsharded inputs, returns FULL
[B,S,D] float32.
"""
import sys
sys.path.insert(0, '/opt/trn_rl_repo')
import numpy as np
from collections import deque

B, S, D = 4, 2048, 2048
H = 16
NOPE, ROPE, VD = 128, 64, 128
QK = NOPE + ROPE
KVR = 512
THETA = 10000.0
EPS = 1e-6
SCALE = QK ** -0.5
NCORES = 8
HPC = 8                    # heads per core
RB = 512                   # row block == attention q block
NRB = S // RB              # 4 blocks per core (one batch)
NQC = 12                   # wq out chunks: 8 nope + 4 pe-pair
NWQ = NRB * NQC            # total wq chunk loads

_cache = {}


def _build_nc():
    import concourse.bass as bass  # noqa: F401
    import concourse.mybir as mybir
    import concourse.tile as tile
    from concourse import bacc
    from contextlib import ExitStack

    F32 = mybir.dt.float32
    F32R = mybir.dt.float32r
    F16 = mybir.dt.float16
    EXP = mybir.ActivationFunctionType.Exp
    LN = mybir.ActivationFunctionType.Ln
    MULT = mybir.AluOpType.mult
    ADD = mybir.AluOpType.add

    nc = bacc.Bacc("TRN2", target_bir_lowering=False, debug=False)

    xT = nc.dram_tensor("xT", [D, S], F16, kind="ExternalInput")
    wqT = nc.dram_tensor("wqT", [128, NQC, 16, 128], F16, kind="ExternalInput")
    wkvT = nc.dram_tensor("wkvT", [D, KVR + 2 * ROPE], F16, kind="ExternalInput")
    wbkT = nc.dram_tensor("wbkT", [KVR, HPC * NOPE], F16, kind="ExternalInput")
    wbvT = nc.dram_tensor("wbvT", [KVR, HPC * VD], F16, kind="ExternalInput")
    woT = nc.dram_tensor("woT", [128, 16, HPC, 128], F16, kind="ExternalInput")
    ropeC = nc.dram_tensor("ropeC", [128, S], F16, kind="ExternalInput")
    ropeS = nc.dram_tensor("ropeS", [128, S], F16, kind="ExternalInput")
    perm = nc.dram_tensor("perm", [128, 128], F16, kind="ExternalInput")
    ones16 = nc.dram_tensor("ones16", [128, 128], F16, kind="ExternalInput")
    onesw = nc.dram_tensor("onesw", [128, 128], F32R, kind="ExternalInput")
    mask0 = nc.dram_tensor("mask0", [128, RB], F16, kind="ExternalInput")
    pout = nc.dram_tensor("pout", [D, S], F16, kind="ExternalOutput")
    pout_r = pout.ap().rearrange("(dg ci) s -> ci dg s", ci=128).rearrange(
        "ci (dq df) s -> ci dq df s", df=2)

    xT_r = xT.ap().rearrange("(co ci) r -> ci co r", ci=128)       # [128,16,S]
    wkvT_r = wkvT.ap().rearrange("(co ci) f -> ci co f", ci=128)   # [128,16,640]
    wbkT_r = wbkT.ap().rearrange("(co ci) f -> ci co f", ci=128)   # [128,4,1024]
    wbvT_r = wbvT.ap().rearrange("(co ci) f -> ci co f", ci=128)

    with tile.TileContext(nc) as tc:
        with ExitStack() as stk:
            gpool = stk.enter_context(tc.tile_pool(name="gconst", bufs=1))
            wp = stk.enter_context(tc.tile_pool(name="pw", bufs=1))
            wqp = stk.enter_context(tc.tile_pool(name="pwq", bufs=3))
            wop = stk.enter_context(tc.tile_pool(name="pwo", bufs=3))
            xp = stk.enter_context(tc.tile_pool(name="px", bufs=1))
            qp_ = stk.enter_context(tc.tile_pool(name="pq", bufs=2))
            sp = stk.enter_context(tc.tile_pool(name="p1sb", bufs=2))
            ep3 = stk.enter_context(tc.tile_pool(name="p3ex", bufs=3))
            sp3 = stk.enter_context(tc.tile_pool(name="p3sb", bufs=1))
            op3 = stk.enter_context(tc.tile_pool(name="p3o", bufs=1))
            sp4 = stk.enter_context(tc.tile_pool(name="p4sb", bufs=2))
            psA = stk.enter_context(tc.tile_pool(name="psproj", bufs=1, space="PSUM"))
            pss = stk.enter_context(tc.tile_pool(name="pssc", bufs=3, space="PSUM"))
            pso = stk.enter_context(tc.tile_pool(name="psacc", bufs=4, space="PSUM"))

            # ---- constants / persistent tiles ----
            perm_sb = gpool.tile([128, 128], F16)
            ones_sb = gpool.tile([128, 128], F16)
            onesr_sb = gpool.tile([128, 128], F32R)
            eps_sb = gpool.tile([128, 1], F32)
            mask_sb = gpool.tile([128, RB], F16)
            kn3 = gpool.tile([128, HPC, S], F16)        # [nope, h, kpos]
            v3 = gpool.tile([128, 16, HPC, VD], F16)    # [kpos128, ktile, h, vd]
            kp3 = gpool.tile([128, S], F16)             # k_pe dup'd halves

            wkv_sb = wp.tile([128, 16, 640], F16)
            wbk_sb = wp.tile([128, 4, HPC * NOPE], F16)
            wbv_sb = wp.tile([128, 4, HPC * VD], F16)
            ropeC_sb = wp.tile([128, S], F16)
            ropeS_sb = wp.tile([128, S], F16)

            # x row-block tile: single-buffered; next block's DMA is issued
            # after the q chains (this block's last readers) in program order.
            xt = xp.tile([128, 16, RB], F16, tag="xt", name="xt0")
            nc.sync.dma_start(xt[:, 0:8, :], xT_r[:, 0:8, 0:RB])
            nc.sync.dma_start(xt[:, 8:16, :], xT_r[:, 8:16, 0:RB])

            # wq chunk stream: chunk g = rb*12 + j, double-buffered
            wq_pend = deque()

            def queue_wq(g):
                t = wqp.tile([128, 16, 128], F16, tag="wqc", name=f"wqc{g}")
                j = g % NQC
                nc.sync.dma_start(t, wqT.ap()[:, j])
                wq_pend.append(t)

            queue_wq(0)
            queue_wq(1)
            queue_wq(2)

            nc.vector.memset(eps_sb, EPS)

            bulk_parts = [
                lambda: nc.sync.dma_start(wkv_sb[:, 0:8, :], wkvT_r[:, 0:8, :]),
                lambda: nc.sync.dma_start(wkv_sb[:, 8:16, :], wkvT_r[:, 8:16, :]),
                lambda: nc.sync.dma_start(wbk_sb, wbkT_r),
                lambda: nc.sync.dma_start(wbv_sb, wbvT_r),
                lambda: (nc.sync.dma_start(perm_sb, perm.ap()),
                         nc.sync.dma_start(ones_sb, ones16.ap()),
                         nc.sync.dma_start(onesr_sb, onesw.ap()),
                         nc.sync.dma_start(mask_sb, mask0.ap())),
                lambda: nc.sync.dma_start(ropeC_sb, ropeC.ap()),
                lambda: nc.sync.dma_start(ropeS_sb, ropeS.ap()),
            ]

            prologue = [True]
            pcnt = [0]

            def proj_ps(name):
                pcnt[0] += 1
                if prologue[0] and pcnt[0] % 2 == 0:
                    return pso.tile([128, RB], F32, tag="o", name=name)
                return psA.tile([128, RB], F32, tag="proj", name=name)

            # wo chunk stream: chunk g = qb*16 + dt, 4 bufs
            wo_pend = deque()

            def queue_wo(g):
                t = wop.tile([128, HPC, 128], F16, tag="woc", name=f"woc{g}")
                dt_ = g % 16
                nc.sync.dma_start(t, woT.ap()[:, dt_])
                wo_pend.append(t)

            for g in range(3):
                queue_wo(g)

            def q_thunks(rb, qn, qp3):
                """q projection chains for row-block rb, as closures."""
                sl = rb * RB
                pend_pe = []

                def chain(j):
                    def f():
                        g = rb * NQC + j
                        wqc = wq_pend.popleft()
                        ps_q = proj_ps("ps_q")
                        for dc in range(16):
                            nc.tensor.matmul(ps_q, wqc[:, dc, :], xt[:, dc, :],
                                             start=(dc == 0), stop=(dc == 15))
                        if g + 3 < NWQ:
                            queue_wq(g + 3)
                        if pend_pe:
                            pend_pe.pop(0)()
                        if j < 8:
                            if j % 2 == 0:
                                nc.scalar.copy(qn[:, j, :], ps_q)
                            else:
                                nc.vector.tensor_copy(qn[:, j, :], ps_q)
                        else:
                            jj = j - 8
                            qpe_sb = sp.tile([128, RB], F16, tag="pe",
                                             name="qpe_sb")
                            nc.scalar.copy(qpe_sb, ps_q)

                            def rope_q(jj=jj, qpe_sb=qpe_sb):
                                ps_qs = pss.tile([128, RB], F32, tag="s",
                                                 name="ps_qs")
                                nc.tensor.matmul(ps_qs, perm_sb, qpe_sb,
                                                 start=True, stop=True)
                                t1 = sp.tile([128, RB], F16, tag="ropt1", bufs=1,
                                             name="t1")
                                nc.vector.tensor_tensor(t1, qpe_sb,
                                                        ropeC_sb[:, sl:sl + RB],
                                                        MULT)
                                nc.vector.tensor_tensor(qp3[:, jj, :], ps_qs,
                                                        ropeS_sb[:, sl:sl + RB],
                                                        MULT)
                                nc.vector.tensor_tensor(qp3[:, jj, :],
                                                        qp3[:, jj, :], t1, ADD)
                            pend_pe.append(rope_q)
                    return f

                def flush():
                    for f in pend_pe:
                        f()
                    pend_pe.clear()
                return [chain(j) for j in range(NQC)] + [flush]

            def kv_thunks(rb):
                """kv latent + k_pe + rmsnorm + k_nope + v chains for block rb,
                as a list of closures to interleave into attention."""
                sl = rb * RB
                st = {}
                thunks = []

                def kv_chain(ft):
                    def f():
                        ps_kv = proj_ps("ps_kv")
                        for dc in range(16):
                            nc.tensor.matmul(ps_kv,
                                             wkv_sb[:, dc, ft * 128:(ft + 1) * 128],
                                             xt[:, dc, :],
                                             start=(dc == 0), stop=(dc == 15))
                        ku = sp.tile([128, RB], F16, tag=f"kvu{ft}", bufs=1,
                                     name=f"ku{ft}")
                        nc.vector.tensor_copy(ku, ps_kv)
                        st.setdefault('kvu', [None] * 4)[ft] = ku
                        if ft == 0:
                            ssacc = sp.tile([128, RB], F32R, tag="ssacc", bufs=1,
                                            name="ssacc")
                            st['ssacc'] = ssacc
                            nc.vector.tensor_tensor(ssacc, ps_kv, ku, MULT)
                        else:
                            sq = sp.tile([128, RB], F16, tag="sq", bufs=1, name="sq")
                            nc.vector.tensor_tensor(sq, ps_kv, ku, MULT)
                            nc.vector.tensor_tensor(st['ssacc'],
                                                    st['ssacc'].bitcast(F32), sq, ADD)
                    return f

                def kpe_chain():
                    ps_kp = proj_ps("ps_kp")
                    for dc in range(16):
                        nc.tensor.matmul(ps_kp, wkv_sb[:, dc, 512:640],
                                         xt[:, dc, :],
                                         start=(dc == 0), stop=(dc == 15))
                    kpe_sb = sp.tile([128, RB], F16, tag="kpe", name="kpe_sb")
                    nc.scalar.copy(kpe_sb, ps_kp)
                    ps_kps = pss.tile([128, RB], F32, tag="s", name="ps_kps")
                    nc.tensor.matmul(ps_kps, perm_sb, kpe_sb, start=True, stop=True)
                    k1 = sp.tile([128, RB], F16, tag="kropt1", bufs=1, name="k1")
                    nc.vector.tensor_tensor(k1, kpe_sb, ropeC_sb[:, sl:sl + RB], MULT)
                    nc.vector.tensor_tensor(kp3[:, sl:sl + RB], ps_kps,
                                            ropeS_sb[:, sl:sl + RB], MULT)
                    nc.vector.tensor_tensor(kp3[:, sl:sl + RB],
                                            kp3[:, sl:sl + RB], k1, ADD)

                def lam_chain():
                    ps_ms = pss.tile([128, RB], F32, tag="s", name="ps_ms")
                    nc.tensor.matmul(ps_ms, onesr_sb, st['ssacc'], start=True,
                                     stop=True)
                    lam = pss.tile([128, RB], F32, tag="s", name="lam")
                    nc.scalar.activation(lam, ps_ms, LN, scale=1.0 / KVR, bias=eps_sb)
                    lam2 = sp.tile([128, RB], F16, tag="lam2", bufs=1, name="lam2")
                    nc.scalar.activation(lam2, lam, EXP, scale=-0.5)
                    for ft in range(4):
                        ku = st['kvu'][ft]
                        nc.vector.tensor_tensor(ku, ku, lam2, MULT)
                    st['kvn'] = st['kvu']

                def kn_chain(h):
                    def f():
                        kvn = st['kvn']
                        ps_k = proj_ps("ps_k")
                        for kc in range(4):
                            nc.tensor.matmul(ps_k,
                                             wbk_sb[:, kc, h * 128:(h + 1) * 128],
                                             kvn[kc], start=(kc == 0), stop=(kc == 3))
                        if h % 2 == 0:
                            nc.vector.tensor_copy(kn3[:, h, sl:sl + RB], ps_k)
                        else:
                            nc.scalar.copy(kn3[:, h, sl:sl + RB], ps_k)
                    return f

                def v_chain(half, rt):
                    def f():
                        kvn = st['kvn']
                        ps_v = proj_ps("ps_v")
                        for kc in range(4):
                            nc.tensor.matmul(
                                ps_v, kvn[kc][:, rt * 128:(rt + 1) * 128],
                                wbv_sb[:, kc, half * 512:(half + 1) * 512],
                                start=(kc == 0), stop=(kc == 3))
                        ro = rb * 4 + rt
                        if (half + rt) % 2 == 0:
                            nc.vector.tensor_copy(
                                v3[:, ro, half * 4:(half + 1) * 4, :], ps_v)
                        else:
                            nc.scalar.copy(
                                v3[:, ro, half * 4:(half + 1) * 4, :], ps_v)
                    return f

                for ft in range(4):
                    thunks.append(kv_chain(ft))
                thunks.append(lam_chain)
                thunks.append(kpe_chain)
                for h in range(HPC):
                    thunks.append(kn_chain(h))
                for half in range(2):
                    for rt in range(4):
                        thunks.append(v_chain(half, rt))
                return thunks

            def p3_thunks(qb, qn, qp3):
                """Attention thunks (per k-tile group) + output projection."""
                nkt = 4 * qb + 4
                o2 = op3.tile([128, HPC, RB], F16, tag="o2", name="o2")
                thunks = []
                st = {}

                def kt_group(hp, kt):
                    def f():
                        hs = hp * 4
                        if kt == 0:
                            st[hp] = {
                                'ps_o': [pso.tile([128, RB], F32, tag="o",
                                                  name=f"ps_o{i}") for i in range(4)],
                                'acc': [sp3.tile([128, RB], F16, tag=f"acc{i}",
                                                 bufs=1, name=f"acc{i}")
                                        for i in range(4)],
                            }
                        ps_o = st[hp]['ps_o']
                        acc = st[hp]['acc']
                        m = kt - 4 * qb
                        lo = max(0, m) * 128
                        ps_s = [pss.tile([128, RB], F32, tag="s", name=f"ps_s{i}")
                                for i in range(4)]
                        for i in range(4):
                            h = hs + i
                            nc.tensor.matmul(ps_s[i][:, lo:],
                                             kn3[:, h, kt * 128:(kt + 1) * 128],
                                             qn[:, h, lo:],
                                             start=True, stop=False)
                        for i in range(4):
                            h = hs + i
                            hb = (h % 2) * 64
                            nc.tensor.matmul(ps_s[i][:, lo:],
                                             kp3[hb:hb + 64, kt * 128:(kt + 1) * 128],
                                             qp3[hb:hb + 64, h // 2, lo:],
                                             start=False, stop=True)
                        for i in range(4):
                            h = hs + i
                            ex = ep3.tile([128, RB], F16, tag="ex", name="ex")
                            nc.scalar.activation(ex[:, lo:], ps_s[i][:, lo:], EXP)
                            if m >= 0:
                                nc.vector.tensor_tensor(
                                    ex[:, lo:], ex[:, lo:],
                                    mask_sb[:, :RB - lo], MULT)
                            nc.tensor.matmul(ps_o[i][:, lo:], v3[:, kt, h, :],
                                             ex[:, lo:],
                                             start=(kt == 0), stop=(kt == nkt - 1))
                            if kt == 0:
                                nc.vector.tensor_copy(acc[i], ex)
                            else:
                                nc.vector.tensor_tensor(acc[i][:, lo:],
                                                        acc[i][:, lo:],
                                                        ex[:, lo:], ADD)
                    return f

                def rowsum(hp):
                    def f():
                        hs = hp * 4
                        ps_o = st[hp]['ps_o']
                        acc = st[hp]['acc']
                        for i in range(4):
                            ps_r = pss.tile([128, RB], F32, tag="s", name="ps_r")
                            nc.tensor.matmul(ps_r, ones_sb, acc[i], start=True,
                                             stop=True)
                            rec = sp3.tile([128, RB], F16, tag="rec", bufs=2,
                                           name="rec")
                            with nc.allow_low_precision(reason="fp16 softmax recip"):
                                nc.vector.reciprocal(rec, ps_r)
                            nc.vector.tensor_tensor(o2[:, hs + i, :], ps_o[i],
                                                    rec, MULT)
                    return f

                def oproj(dt_):
                    def f():
                        g = qb * 16 + dt_
                        woc = wo_pend.popleft()
                        ps_p = pso.tile([128, RB], F32, tag="o", name="ps_p")
                        for hc in range(HPC):
                            nc.tensor.matmul(ps_p, woc[:, hc, :], o2[:, hc, :],
                                             start=(hc == 0), stop=(hc == HPC - 1))
                        if g + 3 < 64:
                            queue_wo(g + 3)
                        if dt_ % 2 == 0:
                            st['po'] = sp4.tile([128, 2, RB], F16, tag="po",
                                                bufs=1, name="po")
                        po = st['po']
                        if dt_ % 2 == 0:
                            nc.scalar.copy(po[:, 0, :], ps_p)
                        else:
                            nc.vector.tensor_copy(po[:, 1, :], ps_p)
                        if dt_ % 2 == 1:
                            nc.sync.dma_start(
                                pout_r[:, dt_ // 2, :,
                                       qb * RB:(qb + 1) * RB], po)
                    return f

                for hp in range(2):
                    for kt in range(nkt):
                        thunks.append(kt_group(hp, kt))
                    thunks.append(rowsum(hp))
                for dt_ in range(16):
                    thunks.append(oproj(dt_))
                return thunks

            def interleave(a, b):
                """Emit thunks of a (attention) and b (next-block projections)
                spread evenly; a's order drives correctness, b fills PE."""
                na, nb = len(a), len(b)
                j = 0
                for i, fa in enumerate(a):
                    fa()
                    want = (i + 1) * nb // na
                    while j < want:
                        b[j]()
                        j += 1
                while j < nb:
                    b[j]()
                    j += 1

            # ---- main schedule: block 0 serial, then everything for
            # block rb+1 (q, kv, x refill) interleaved into attention rb ----
            def xt_refill(rb):
                sl = rb * RB
                nc.sync.dma_start(xt[:, 0:8, :], xT_r[:, 0:8, sl:sl + RB])
                nc.sync.dma_start(xt[:, 8:16, :], xT_r[:, 8:16, sl:sl + RB])

            def alloc_q():
                qn = qp_.tile([128, HPC, RB], F16, tag="qn", name="qn")
                qp3 = qp_.tile([128, 4, RB], F16, tag="qp", name="qp")
                return qn, qp3

            q_tiles = {0: alloc_q()}
            qt0 = q_thunks(0, *q_tiles[0])
            for i, f in enumerate(qt0):
                f()
                if i < len(bulk_parts):
                    bulk_parts[i]()
            for f in kv_thunks(0):
                f()
            xt_refill(1)
            prologue[0] = False
            for rb in range(NRB):
                b = []
                if rb + 1 < NRB:
                    q_tiles[rb + 1] = alloc_q()
                    b = (q_thunks(rb + 1, *q_tiles[rb + 1])
                         + kv_thunks(rb + 1))
                    if rb + 2 < NRB:
                        b.append(lambda rb=rb: xt_refill(rb + 2))
                interleave(p3_thunks(rb, *q_tiles.pop(rb)), b)

    nc.compile()
    return nc


def _prep_inputs(x, wq, wkv_a, kv_norm_w, wkv_b, wo, freqs_cos, freqs_sin):
    f16 = np.float16
    x = np.asarray(x, np.float32)
    wq = np.asarray(wq, np.float32)
    wkv_a = np.asarray(wkv_a, np.float32)
    kv_norm_w = np.asarray(kv_norm_w, np.float32)
    wkv_b = np.asarray(wkv_b, np.float32)
    wo = np.asarray(wo, np.float32)
    cos = np.asarray(freqs_cos, np.float32)   # [S, 32]
    sin = np.asarray(freqs_sin, np.float32)

    C64 = np.repeat(cos.T, 2, axis=0)         # [64, S]
    S64 = np.repeat(sin.T, 2, axis=0).copy()
    S64[0::2] *= -1.0                         # even rows: -sin; odd: +sin
    ropeC = np.ascontiguousarray(np.vstack([C64, C64])).astype(f16)   # [128,S]
    ropeS = np.ascontiguousarray(np.vstack([S64, S64])).astype(f16)

    perm = np.zeros((128, 128), np.float32)
    idx = np.arange(128)
    perm[idx ^ 1, idx] = 1.0                  # out[m] = in[m^1]
    ones = np.ones((128, 128), np.float32)

    # single causal mask tile: mask0[p, c] = 1.0 iff c >= p
    mask0 = (np.arange(RB)[None, :] >= np.arange(128)[:, None]).astype(f16)

    wq_h = wq.reshape(H, QK, D)
    wb_h = (wkv_b * kv_norm_w[None, :]).reshape(H, NOPE + VD, KVR)
    wkv_prep = np.concatenate([wkv_a, wkv_a[KVR:]], axis=0)  # [640, D] dup'd pe

    xT_b = [np.ascontiguousarray(x[b].T).astype(f16) for b in range(B)]
    wkvT = np.ascontiguousarray(wkv_prep.T).astype(f16)

    in_maps = []
    for c in range(NCORES):
        b, hg = c // 2, c % 2
        heads = list(range(hg * HPC, (hg + 1) * HPC))
        # q chunks: 8 per-head nope blocks, then 4 pe pair blocks
        qrows = [wq_h[h, :NOPE] for h in heads]
        for j in range(4):
            qrows.append(wq_h[heads[2 * j], NOPE:])
            qrows.append(wq_h[heads[2 * j + 1], NOPE:])
        wq_prep = np.concatenate(qrows, axis=0) * SCALE          # [1536, D]
        wbk = np.concatenate([wb_h[h, :NOPE] for h in heads], axis=0)   # [1024,512]
        wbv = np.concatenate([wb_h[h, NOPE:] for h in heads], axis=0)
        wo_c = np.concatenate([wo[:, h * VD:(h + 1) * VD] for h in heads],
                              axis=1)                            # [D, 1024]
        in_maps.append({
            "xT": xT_b[b],
            "wqT": np.ascontiguousarray(
                wq_prep.reshape(NQC, 128, 16, 128).transpose(3, 0, 2, 1)
            ).astype(f16),
            "wkvT": wkvT,
            "wbkT": np.ascontiguousarray(wbk.T).astype(f16),
            "wbvT": np.ascontiguousarray(wbv.T).astype(f16),
            "woT": np.ascontiguousarray(
                wo_c.reshape(16, 128, HPC, 128).transpose(3, 0, 2, 1)
            ).astype(f16),
            "ropeC": ropeC,
            "ropeS": ropeS,
            "perm": perm.astype(f16),
            "ones16": ones.astype(f16),
            "onesw": ones.astype(np.float32),
            "mask0": mask0,
        })
    return in_maps


def _get_nc():
    if "nc" not in _cache:
        _cache["nc"] = _build_nc()
    return _cache["nc"]


def kernel(**inputs):
    from concourse.bass_utils import run_bass_kernel_spmd
    nc = _get_nc()
    in_maps = _prep_inputs(**inputs)
    res = run_bass_kernel_spmd(nc, in_maps, core_ids=list(range(NCORES)))
    out = np.empty((B, S, D), np.float32)
    for b in range(B):
        acc = res.results[2 * b]["pout"].astype(np.float32)
        acc += res.results[2 * b + 1]["pout"].astype(np.float32)
        out[b] = acc.T
    return out
